# revision 19
# baseline (speedup 1.0000x reference)
"""Trainium2 Bass kernel for the unrolled-GRU + FC-head problem.

Math (per example b):
    gi[t] = x[t] @ w_ih.T + b_ih                       # [T, 3H]
    gh    = h  @ w_hh.T + b_hh                         # per step
    r = sig(gi_r + gh_r); z = sig(gi_z + gh_z)
    n = tanh(gi_n + r * gh_n)
    h = (1 - z) * n + z * h                            # T sequential steps
    out = relu(h @ w_fc1.T + b_fc1) @ w_fc2.T + b_fc2  # [C]

Sharding: data-parallel over batch. B=512 over 8 cores -> B_local=64.

Per-core design (f32r matmuls, batch stationary M=64, weights moving):
  - PSUM G [64,1536] (r,z): biases + x-proj + h-proj accumulated
  - PSUM Gin [64,768]: b_ih(n) + x-proj(n);  Ghn [64,768]: b_hh(n) + h-proj(n)
  - per-step PE streaming: 13824 (h-proj) + 2304 (x-proj) + 3072 (bias)
    + 768 (transposes) cycles; the elementwise tail must hide under it.

v3 tail (vs the 2481600ns baseline):
  - halves-pipelined tail: per half: sig r -> tn -> tn2 -> tanh -> w1 -> h,
    with w1 = (z-1)*n fused in ONE DVE op (scalar_tensor_tensor), and
    u = z*h on the idle GpSimd engine (SBUF-only operands).
  - transposes interleaved per half: T0-2 right after h half 1, so the PE
    restarts h-proj(t+1) ~a half-tail earlier.
  - hT PSUM->SBUF copies moved to ACT (DVE was the second-busiest engine).
  - emission order keeps bias/x-proj of t+1 ahead of the transposes in the
    PE FIFO so the PE never idles long enough to re-throttle (HAM).
"""

import os
import sys

import numpy as np

if "/opt/trn_rl_repo" not in sys.path:
    sys.path.insert(0, "/opt/trn_rl_repo")

B, T, I, H, F1, C = 512, 128, 128, 768, 256, 10
NCORES = 8
BL = B // NCORES  # 64
G3 = 3 * H  # 2304
H2 = 2 * H  # 1536
KC = H // 128  # 6 k-chunks of the hidden dim

HALVES = int(os.environ.get("GRU_HALVES", "2"))  # tail column chunks
UGP = os.environ.get("GRU_UGP", "1") == "1"  # u = z*h on gpsimd
CPACT = os.environ.get("GRU_CPACT", "1") == "1"  # hT copies on ACT

_CACHE = {}


def _build_program(reps=1):
    import concourse.bacc as bacc
    import concourse.mybir as mybir
    import concourse.tile as tile
    from concourse.masks import make_identity

    f32 = mybir.dt.float32
    f32r = mybir.dt.float32r
    AF = mybir.ActivationFunctionType
    ALU = mybir.AluOpType

    nc = bacc.Bacc(
        "TRN2",
        target_bir_lowering=False,
        debug=False,
        enable_asserts=False,
        num_devices=NCORES,
    )

    # ---- DRAM I/O (f32r tensors carry plain fp32 bytes from numpy) ----
    xT_d = nc.dram_tensor("xT", [128, T * BL], f32r, kind="ExternalInput")
    whhT_d = nc.dram_tensor("whhT", [128, KC * G3], f32r, kind="ExternalInput")
    wihT_d = nc.dram_tensor("wihT", [128, G3], f32r, kind="ExternalInput")
    brz_d = nc.dram_tensor("brz", [1, H2], f32r, kind="ExternalInput")
    bin_d = nc.dram_tensor("bin", [1, H], f32r, kind="ExternalInput")
    bhn_d = nc.dram_tensor("bhn", [1, H], f32r, kind="ExternalInput")
    ones_d = nc.dram_tensor("ones", [1, BL], f32r, kind="ExternalInput")
    wfc1T_d = nc.dram_tensor("wfc1T", [128, KC * F1], f32r, kind="ExternalInput")
    bfc1_d = nc.dram_tensor("bfc1", [1, F1], f32r, kind="ExternalInput")
    wfc2T_d = nc.dram_tensor("wfc2T", [128, 2 * C], f32r, kind="ExternalInput")
    bfc2_d = nc.dram_tensor("bfc2", [1, C], f32r, kind="ExternalInput")
    out_d = nc.dram_tensor("logits", [BL, C], f32, kind="ExternalOutput")

    with tile.TileContext(nc) as tc:
        with (
            tc.tile_pool(name="const", bufs=1) as const,
            tc.tile_pool(name="state", bufs=2) as state,
            tc.tile_pool(name="work", bufs=2) as work,
            tc.tile_pool(name="gpsum", bufs=1, space="PSUM") as gpsum,
        ):
            # ---- constants: DMA everything in once ----
            def load(name, shape, dram):
                t_ = const.tile(shape, f32r, tag=name)
                nc.sync.dma_start(out=t_[:], in_=dram.ap())
                return t_

            xT = load("xT", [128, T * BL], xT_d)
            wihT = load("wihT", [128, G3], wihT_d)
            whhT = load("whhT", [128, KC * G3], whhT_d)
            brz = load("brz", [1, H2], brz_d)
            bin_ = load("bin", [1, H], bin_d)
            bhn = load("bhn", [1, H], bhn_d)
            ones = load("ones", [1, BL], ones_d)
            wfc1T = load("wfc1T", [128, KC * F1], wfc1T_d)
            bfc1 = load("bfc1", [1, F1], bfc1_d)
            wfc2T = load("wfc2T", [128, 2 * C], wfc2T_d)
            bfc2 = load("bfc2", [1, C], bfc2_d)

            ident = const.tile([BL, BL], f32, tag="ident")
            make_identity(nc, ident[:])

            h_prev = None  # SBUF [64, 768] fp32
            hT = None  # SBUF [128, KC*64] f32r (transposed h)

            mm = nc.tensor.matmul

            def emit_transposes(h_sb, Tps, ks):
                # PE transposes of 128-col blocks of h -> hT chunks ks
                for k in ks:
                    nc.tensor.transpose(
                        Tps[:, k * BL : (k + 1) * BL],
                        h_sb[:, k * 128 : (k + 1) * 128],
                        ident[:],
                    )

            def emit_copy(hT_new, Tps, c0, c1):
                eng = nc.scalar if CPACT else nc.vector
                if CPACT:
                    nc.scalar.activation(hT_new[:, c0:c1], Tps[:, c0:c1], AF.Copy)
                else:
                    nc.vector.tensor_copy(hT_new[:, c0:c1], Tps[:, c0:c1])

            CH = ((0, 512), (512, 768))  # 768-col regions in 2 psum chunks

            def emit_matmuls_rn(t):
                """bias + x-proj for r/n regions (z emitted separately,
                LAST, so the r/n tail of step t-1 unblocks these banks)."""
                Gr = gpsum.tile([BL, H], f32, tag="Gr")
                Gin = gpsum.tile([BL, H], f32, tag="Gin")
                Ghn = gpsum.tile([BL, H], f32, tag="Ghn")
                xt = xT[:, t * BL : (t + 1) * BL]

                # emission order = unblock order: Gr banks free first (the
                # r sigmoid reads them early in the z phase of step t-1),
                # then Gin bank0 (tn2 chunk0), Ghn, Gin bank1.
                for c0, c1 in CH:
                    mm(Gr[:, c0:c1], ones[:], brz[:, c0:c1],
                       start=True, stop=False)
                for c0, c1 in CH:
                    mm(Gr[:, c0:c1], xt, wihT[:, c0:c1],
                       start=False, stop=(t == 0))
                mm(Gin[:, 0:512], ones[:], bin_[:, 0:512],
                   start=True, stop=False)
                mm(Gin[:, 0:512], xt, wihT[:, H2 : H2 + 512],
                   start=False, stop=True)
                for c0, c1 in CH:
                    mm(Ghn[:, c0:c1], ones[:], bhn[:, c0:c1],
                       start=True, stop=(t == 0))
                mm(Gin[:, 512:768], ones[:], bin_[:, 512:768],
                   start=True, stop=False)
                mm(Gin[:, 512:768], xt, wihT[:, H2 + 512 : H2 + 768],
                   start=False, stop=True)
                return Gr, Gin, Ghn

            def emit_hproj_k(Gr, Ghn, hT_cur, k):
                hk = hT_cur[:, k * BL : (k + 1) * BL]
                wk = k * G3
                last = k == KC - 1
                # r before hn everywhere: the r sigmoid is the head of
                # the tail chain, so its last chunk should land first.
                for c0, c1 in CH:
                    mm(Gr[:, c0:c1], hk, whhT[:, wk + c0 : wk + c1],
                       start=False, stop=last)
                for c0, c1 in CH:
                    mm(Ghn[:, c0:c1], hk,
                       whhT[:, wk + H2 + c0 : wk + H2 + c1],
                       start=False, stop=last)

            def emit_z(t, hT_cur):
                """bias + x-proj + h-proj for the z gate, streamed last."""
                Gz = gpsum.tile([BL, H], f32, tag="Gz")
                xt = xT[:, t * BL : (t + 1) * BL]
                for c0, c1 in CH:
                    mm(Gz[:, c0:c1], ones[:], brz[:, H + c0 : H + c1],
                       start=True, stop=False)
                for c0, c1 in CH:
                    mm(Gz[:, c0:c1], xt, wihT[:, H + c0 : H + c1],
                       start=False, stop=(t == 0))
                if t > 0:
                    for k in range(KC):
                        hk = hT_cur[:, k * BL : (k + 1) * BL]
                        wk = k * G3
                        last = k == KC - 1
                        # at k5 stream the (512:768) chunk last so the
                        # first z-sigmoid chunk unblocks ~300ns earlier
                        ch = reversed(CH) if last else CH
                        for c0, c1 in ch:
                            mm(Gz[:, c0:c1], hk,
                               whhT[:, wk + H + c0 : wk + H + c1],
                               start=False, stop=last)
                return Gz

            def emit_tail_rn(t, Gr, Gin, Ghn, sl):
                """r sigmoid -> tn -> tn2 -> tanh (+ d chunk0); runs during
                the z stream of step t (pre-z-end). DVE order frees Gin
                bank0 first so next-step bias can start."""
                r_s = work.tile([BL, H], f32, tag="r")
                tn = work.tile([BL, H], f32, tag="tn")
                tn2 = work.tile([BL, H], f32, tag="tn2")
                n_t = work.tile([BL, H], f32, tag="n")
                d = work.tile([BL, H], f32, tag="d")
                for s in sl:
                    nc.scalar.activation(r_s[:, s], Gr[:, s], AF.Sigmoid)
                for s in sl:
                    nc.vector.tensor_mul(tn[:, s], r_s[:, s], Ghn[:, s])
                    nc.vector.tensor_add(tn2[:, s], tn[:, s], Gin[:, s])
                for s in sl:
                    nc.scalar.activation(n_t[:, s], tn2[:, s], AF.Tanh)
                # d = h - n (h = n + z*d); chunk0 now, chunk1 emitted late
                # (just before the last m) so a slow tanh1 can't block the
                # early m/h chunks in the DVE FIFO
                s = sl[0]
                if t > 0:
                    nc.vector.tensor_sub(d[:, s], h_prev[:, s], n_t[:, s])
                else:
                    nc.vector.tensor_scalar_mul(d[:, s], n_t[:, s], -1.0)
                return n_t, d

            def emit_d1(t, n_t, d, s):
                if t > 0:
                    nc.vector.tensor_sub(d[:, s], h_prev[:, s], n_t[:, s])
                else:
                    nc.vector.tensor_scalar_mul(d[:, s], n_t[:, s], -1.0)

            def emit_tail_z(t, Gz, n_t, d, slz, d1s):
                """per 256-chunk: z sigmoid -> m = z*d -> h = n + m."""
                h_new = state.tile([BL, H], f32, tag="h")
                z_s = work.tile([BL, H], f32, tag="z")
                m = work.tile([BL, H], f32, tag="m")
                for ci, s in enumerate(slz):
                    nc.scalar.activation(z_s[:, s], Gz[:, s], AF.Sigmoid)
                for ci, s in enumerate(slz):
                    if ci == len(slz) - 1:
                        emit_d1(t, n_t, d, d1s)
                    nc.vector.tensor_mul(m[:, s], z_s[:, s], d[:, s])
                    nc.vector.tensor_add(h_new[:, s], n_t[:, s], m[:, s])
                return h_new

            def emit_step(t):
                nonlocal h_prev, hT
                sl = [slice(0, 512), slice(512, H)]

                Gr, Gin, Ghn = emit_matmuls_rn(t)

                if t > 0:
                    # transposes grouped by the h column ranges the tail
                    # produced (512 then 256 cols), interleaved with their
                    # dependent h-proj k-chunks so a not-yet-ready
                    # transpose never blocks ready h-proj work in the FIFO
                    Tps = gpsum.tile([128, KC * BL], f32, tag="Gz")
                    hT_new = state.tile([128, KC * BL], f32r, tag="hT")
                    emit_transposes(h_prev, Tps, (0, 1, 2, 3))
                    emit_copy(hT_new, Tps, 0, 256)
                    for k in (0, 1, 2, 3):
                        emit_hproj_k(Gr, Ghn, hT_new, k)
                    emit_transposes(h_prev, Tps, (4, 5))
                    emit_copy(hT_new, Tps, 256, 384)
                    for k in (4, 5):
                        emit_hproj_k(Gr, Ghn, hT_new, k)
                    hT = hT_new

                Gz = emit_z(t, hT)
                n_t, d = emit_tail_rn(t, Gr, Gin, Ghn, sl)
                h_prev = emit_tail_z(t, Gz, n_t, d, sl, sl[1])

            def emit_fc_head():
                nonlocal h_prev, hT
                Tps = gpsum.tile([128, KC * BL], f32, tag="Gz")
                hT_new = state.tile([128, KC * BL], f32r, tag="hT")
                emit_transposes(h_prev, Tps, range(KC))
                emit_copy(hT_new, Tps, 0, KC * BL)
                hT = hT_new

                fc1 = gpsum.tile([BL, F1], f32, tag="Gr")
                mm(fc1[:], ones[:], bfc1[:], start=True, stop=False)
                for k in range(KC):
                    mm(fc1[:], hT[:, k * BL : (k + 1) * BL],
                       wfc1T[:, k * F1 : (k + 1) * F1],
                       start=False, stop=(k == KC - 1))
                o1 = work.tile([BL, F1], f32, tag="o1")
                nc.scalar.activation(o1[:], fc1[:], AF.Relu)

                T2 = gpsum.tile([128, 2 * BL], f32, tag="Gz")
                nc.tensor.transpose(T2[:, 0:BL], o1[:, 0:128], ident[:])
                nc.tensor.transpose(T2[:, BL : 2 * BL], o1[:, 128:256], ident[:])
                o1T = work.tile([128, 2 * BL], f32r, tag="o1T")
                nc.vector.tensor_copy(o1T[:], T2[:])

                fc2 = gpsum.tile([BL, C], f32, tag="Gin")
                mm(fc2[:], ones[:], bfc2[:], start=True, stop=False)
                mm(fc2[:], o1T[:, 0:BL], wfc2T[:, 0:C], start=False, stop=False)
                mm(fc2[:], o1T[:, BL : 2 * BL], wfc2T[:, C : 2 * C],
                   start=False, stop=True)
                lo = work.tile([BL, C], f32, tag="lo")
                nc.vector.tensor_copy(lo[:], fc2[:])
                nc.sync.dma_start(out=out_d.ap(), in_=lo[:])

            def emit_body():
                for t in range(T):
                    emit_step(t)
                emit_fc_head()

            if reps > 1:
                with tc.For_i(0, reps, 1):
                    emit_body()
            else:
                emit_body()

    nc.compile()
    return nc


def _prep_shared(w_ih, w_hh, b_ih, b_hh, w_fc1, b_fc1, w_fc2, b_fc2):
    f = np.float32

    def kmajor(wT, kc, n):  # [kc*128, n] -> [128, kc*n]
        return np.ascontiguousarray(
            wT.reshape(kc, 128, n).transpose(1, 0, 2).reshape(128, kc * n)
        ).astype(f, copy=False)

    whhT = kmajor(np.ascontiguousarray(w_hh.T), KC, G3)
    wihT = np.ascontiguousarray(w_ih.T).astype(f, copy=False)
    b_sum = (b_ih + b_hh).astype(f)
    shared = {
        "whhT": whhT,
        "wihT": wihT,
        "brz": np.ascontiguousarray(b_sum[None, :H2]),
        "bin": np.ascontiguousarray(b_ih.astype(f)[None, H2:G3]),
        "bhn": np.ascontiguousarray(b_hh.astype(f)[None, H2:G3]),
        "ones": np.ones((1, BL), f),
        "wfc1T": kmajor(np.ascontiguousarray(w_fc1.T), KC, F1),
        "bfc1": np.ascontiguousarray(b_fc1.astype(f)[None, :]),
        "wfc2T": kmajor(np.ascontiguousarray(w_fc2.T), 2, C),
        "bfc2": np.ascontiguousarray(b_fc2.astype(f)[None, :]),
    }
    return shared


def _prep_in_maps(inputs):
    x = np.asarray(inputs["x"], dtype=np.float32)
    shared = _prep_shared(
        *(np.asarray(inputs[k], dtype=np.float32)
          for k in ("w_ih", "w_hh", "b_ih", "b_hh", "w_fc1", "b_fc1",
                    "w_fc2", "b_fc2"))
    )
    in_maps = []
    for c in range(NCORES):
        xs = x[c * BL : (c + 1) * BL]  # [64, T, I]
        xT = np.ascontiguousarray(xs.transpose(2, 1, 0).reshape(128, T * BL))
        in_maps.append({**shared, "xT": xT})
    return in_maps


def _execute(in_maps, reps=1):
    from concourse.bass_utils import run_bass_kernel_spmd

    key = ("nc", reps)
    if key not in _CACHE:
        _CACHE[key] = _build_program(reps=reps)
    nc = _CACHE[key]
    res = run_bass_kernel_spmd(nc, in_maps, core_ids=list(range(NCORES)))
    out = np.concatenate([res.results[c]["logits"] for c in range(NCORES)], axis=0)
    return out.astype(np.float32), res


def _run(inputs, trace=False, trace_kwargs=None):
    return _execute(_prep_in_maps(inputs))


def kernel(**inputs):
    out, _ = _execute(_prep_in_maps(inputs))
    return out


# revision 20
# speedup vs baseline: 1.3267x; 1.3267x over previous
"""Trainium2 Bass kernel for the unrolled-GRU + FC-head problem.

Math (per example b):
    gi[t] = x[t] @ w_ih.T + b_ih                       # [T, 3H]
    gh    = h  @ w_hh.T + b_hh                         # per step
    r = sig(gi_r + gh_r); z = sig(gi_z + gh_z)
    n = tanh(gi_n + r * gh_n)
    h = (1 - z) * n + z * h                            # T sequential steps
    out = relu(h @ w_fc1.T + b_fc1) @ w_fc2.T + b_fc2  # [C]

Sharding: data-parallel over batch. B=512 over 8 cores -> B_local=64.

Per-core design (matmul operands are float32r -- full-rate 1 col/cycle on
the PE for N>=256, measured ~7e-5 relative error, far better than tf32):
  - batch is the matmul *stationary* operand: lhsT = h^T chunk [K=128, M=64],
    weights stream as the moving operand (rhs = w^T [128, N<=512]).
  - PSUM G    [64,1536]: b_ih+b_hh (r,z) + x-proj + h-proj accumulated
  - PSUM Gin  [64, 768]: b_ih(n) + x-proj(n)
  - PSUM Ghn  [64, 768]: b_hh(n) + h-proj(n)
  - biases folded into PSUM via rank-1 matmuls (ones[1,64] stationary).
  - h^T for the next step produced by 6 PE transposes + one DVE copy
    (the copy also performs the f32 -> f32r rounding the verifier needs).
"""

import os
import sys

import numpy as np

if "/opt/trn_rl_repo" not in sys.path:
    sys.path.insert(0, "/opt/trn_rl_repo")

B, T, I, H, F1, C = 512, 128, 128, 768, 256, 10
NCORES = 8
BL = B // NCORES  # 64
G3 = 3 * H  # 2304
H2 = 2 * H  # 1536
KC = H // 128  # 6 k-chunks of the hidden dim

# v2 experiment knobs
COLTILE = os.environ.get("GRU_COLTILE", "0") == "1"
TAILOPT = os.environ.get("GRU_TAILOPT", "1") == "1"
# dummy matmuls per step to keep the PE HAM clock-gate warm during the
# elementwise tail (each ~160ns of PE work into the scratch transpose bank)
FILLER = int(os.environ.get("GRU_FILLER", "0"))
# split the elementwise chain into H-halves so the first half's h/hT feeds
# the PE while the second half is still being computed
HALVES = int(os.environ.get("GRU_HALVES", "1"))
# timing-bisect knobs (produce WRONG results; for attribution only)
SKIP_ELEM = os.environ.get("GRU_SKIP_ELEM", "0") == "1"
SKIP_H = os.environ.get("GRU_SKIP_H", "0") == "1"
# bf16 h-matmuls with K-split column pairing (even k -> psum partitions
# 0:64, odd k -> 64:128, streamed concurrently; DVE folds the partials)
BF16H = os.environ.get("GRU_BF16H", "0") == "1"

_CACHE = {}


def _build_program(reps=1):
    import contextlib

    import concourse.bacc as bacc
    import concourse.mybir as mybir
    import concourse.tile as tile
    from concourse.masks import make_identity

    f32 = mybir.dt.float32
    f32r = mybir.dt.float32r
    AF = mybir.ActivationFunctionType

    nc = bacc.Bacc(
        "TRN2",
        target_bir_lowering=False,
        debug=False,
        enable_asserts=False,
        num_devices=NCORES,
    )

    def mm(out, lhsT, rhs, start, stop):
        """Matmul with batch (M=64) as stationary. With COLTILE, split the
        batch into two 32-column groups of the PE array: the two matmuls
        stream concurrently on separate XBUSes (disjoint output partitions),
        halving the weight-streaming wall time."""
        if not COLTILE:
            nc.tensor.matmul(out, lhsT, rhs, start=start, stop=stop)
            return
        hb = BL // 2
        nc.tensor.matmul(out[0:hb, :], lhsT[:, 0:hb], rhs,
                         start=start, stop=stop)
        nc.tensor.matmul(out[hb:BL, :], lhsT[:, hb:BL], rhs,
                         start=start, stop=stop)

    # ---- DRAM I/O (f32r tensors carry plain fp32 bytes from numpy) ----
    xT_d = nc.dram_tensor("xT", [128, T * BL], f32r, kind="ExternalInput")
    bf16 = mybir.dt.bfloat16
    whh_dt = bf16 if BF16H else f32r
    whhT_d = nc.dram_tensor("whhT", [128, KC * G3], whh_dt, kind="ExternalInput")
    wihT_d = nc.dram_tensor("wihT", [128, G3], f32r, kind="ExternalInput")
    brz_d = nc.dram_tensor("brz", [1, H2], f32r, kind="ExternalInput")
    bin_d = nc.dram_tensor("bin", [1, H], f32r, kind="ExternalInput")
    bhn_d = nc.dram_tensor("bhn", [1, H], f32r, kind="ExternalInput")
    ones_d = nc.dram_tensor("ones", [1, BL], f32r, kind="ExternalInput")
    wfc1T_d = nc.dram_tensor("wfc1T", [128, KC * F1], f32r, kind="ExternalInput")
    bfc1_d = nc.dram_tensor("bfc1", [1, F1], f32r, kind="ExternalInput")
    wfc2T_d = nc.dram_tensor("wfc2T", [128, 2 * C], f32r, kind="ExternalInput")
    bfc2_d = nc.dram_tensor("bfc2", [1, C], f32r, kind="ExternalInput")
    out_d = nc.dram_tensor("logits", [BL, C], f32, kind="ExternalOutput")

    with tile.TileContext(nc) as tc:
        with (
            tc.tile_pool(name="const", bufs=1) as const,
            tc.tile_pool(name="state", bufs=2) as state,
            tc.tile_pool(name="work", bufs=2) as work,
            tc.tile_pool(name="gpsum", bufs=1, space="PSUM") as gpsum,
            tc.tile_pool(name="tpsum", bufs=1, space="PSUM") as tpsum,
        ):
            # ---- constants: DMA everything in once ----
            def load(name, shape, dram):
                t_ = const.tile(shape, f32r, tag=name)
                nc.sync.dma_start(out=t_[:], in_=dram.ap())
                return t_

            xT = load("xT", [128, T * BL], xT_d)
            wihT = load("wihT", [128, G3], wihT_d)
            brz = load("brz", [1, H2], brz_d)
            bin_ = load("bin", [1, H], bin_d)
            bhn = load("bhn", [1, H], bhn_d)
            ones = load("ones", [1, BL], ones_d)
            whhT = const.tile([128, KC * G3], whh_dt, tag="whhT")
            nc.sync.dma_start(out=whhT[:], in_=whhT_d.ap())
            wfc1T = load("wfc1T", [128, KC * F1], wfc1T_d)
            bfc1 = load("bfc1", [1, F1], bfc1_d)
            wfc2T = load("wfc2T", [128, 2 * C], wfc2T_d)
            bfc2 = load("bfc2", [1, C], bfc2_d)

            ident = const.tile([BL, BL], f32, tag="ident")
            make_identity(nc, ident[:])

            h_prev = None  # SBUF [64, 768] fp32
            hT = None  # SBUF [128, KC*64] f32r (transposed h)

            def transpose_h(h_sb, hT_dt=None):
                # 6 PE transposes; PSUM->SBUF copies per half so the first
                # half of hT is available while the rest transposes.
                if hT_dt is None:
                    hT_dt = bf16 if BF16H else f32r
                Tps = tpsum.tile([128, KC * BL], f32, tag="T")
                hT_new = state.tile([128, KC * BL], hT_dt, tag="hT")
                half = KC * BL // 2  # 192
                for k in range(KC):
                    nc.tensor.transpose(
                        Tps[:, k * BL : (k + 1) * BL],
                        h_sb[:, k * 128 : (k + 1) * 128],
                        ident[:],
                    )
                    if k == KC // 2 - 1:
                        nc.vector.tensor_copy(hT_new[:, 0:half], Tps[:, 0:half])
                nc.vector.tensor_copy(hT_new[:, half:], Tps[:, half:])
                return hT_new

            def emit_body():
                emit_recurrence()
                emit_fc_head()

            def emit_recurrence():
                nonlocal h_prev, hT
                for t in range(T):
                    emit_step(t)

            def emit_step(t):
                nonlocal h_prev, hT
                gp = 128 if BF16H else BL
                G = gpsum.tile([gp, H2], f32, tag="G")
                Gin = gpsum.tile([BL, H], f32, tag="Gin")
                Ghn = gpsum.tile([gp, H], f32, tag="Ghn")
                G0, Ghn0 = G[0:BL, :], Ghn[0:BL, :]
                xt = xT[:, t * BL : (t + 1) * BL]

                # -- PE: bias init (start=True claims each bank) --
                for c0 in range(0, H2, 512):
                    mm(G0[:, c0 : c0 + 512], ones[:], brz[:, c0 : c0 + 512],
                       start=True, stop=False)
                for c0, c1 in ((0, 512), (512, 768)):
                    mm(Gin[:, c0:c1], ones[:], bin_[:, c0:c1],
                       start=True, stop=False)
                    mm(Ghn0[:, c0:c1], ones[:], bhn[:, c0:c1],
                       start=True, stop=(t == 0 or SKIP_H))

                # -- PE: x projection --
                for c0 in range(0, H2, 512):
                    mm(G0[:, c0 : c0 + 512], xt, wihT[:, c0 : c0 + 512],
                       start=False, stop=(t == 0 or SKIP_H))
                for c0, c1 in ((0, 512), (512, 768)):
                    mm(Gin[:, c0:c1], xt, wihT[:, H2 + c0 : H2 + c1],
                       start=False, stop=True)

                if FILLER and t > 0:
                    # dummy PE work into the scratch transpose bank; keeps
                    # the HAM activity window busy while ACT/DVE finish the
                    # previous step's gates. Results are never read.
                    Fps = tpsum.tile([128, KC * BL], f32, tag="T")
                    for _ in range(FILLER):
                        nc.tensor.matmul(Fps[0:64, :], xt,
                                         whhT[:, 0 : KC * BL],
                                         start=True, stop=True)

                if t > 0 and not SKIP_H:
                    # -- PE: transpose h_{t-1} -> hT, then h projection --
                    hT = transpose_h(h_prev)
                    for k in range(KC):
                        hk = hT[:, k * BL : (k + 1) * BL]
                        wk = k * G3
                        if BF16H:
                            odd = k % 2 == 1
                            Gt = G[64:128, :] if odd else G0
                            Ghnt = Ghn[64:128, :] if odd else Ghn0
                            st = odd and k == 1  # odd chain opens at k=1
                            last = k >= KC - 2
                        else:
                            Gt, Ghnt, st = G0, Ghn0, False
                            last = k == KC - 1

                        def g_mms():
                            for c0 in range(0, H2, 512):
                                mm(Gt[:, c0 : c0 + 512], hk,
                                   whhT[:, wk + c0 : wk + c0 + 512],
                                   start=st, stop=last)

                        def hn_mms():
                            for c0, c1 in ((0, 512), (512, 768)):
                                mm(Ghnt[:, c0:c1], hk,
                                   whhT[:, wk + H2 + c0 : wk + H2 + c1],
                                   start=st, stop=last)

                        # last k-group: finish Ghn and the r-region chunks
                        # first so the r-sigmoid / tn chain starts earlier
                        # (the z-region chunk [1024:1536] stops last).
                        if last:
                            hn_mms()
                            g_mms()
                        else:
                            g_mms()
                            hn_mms()

                # -- ACT/DVE: gates + state update --
                h_new = state.tile([BL, H], f32, tag="h")
                if SKIP_ELEM:
                    nc.vector.tensor_copy(h_new[:], G[0:BL, 0:H])
                    h_prev = h_new
                    return
                elif TAILOPT:
                    if BF16H and t > 0:
                        gf = work.tile([BL, H2], f32, tag="gf")
                        hnf = work.tile([BL, H], f32, tag="hnf")
                        nc.vector.tensor_add(gf[:, 0:H], G[0:BL, 0:H],
                                             G[64:128, 0:H])
                        nc.vector.tensor_add(gf[:, H:H2], G[0:BL, H:H2],
                                             G[64:128, H:H2])
                        nc.vector.tensor_add(hnf[:], Ghn[0:BL, :],
                                             Ghn[64:128, :])
                        Gv, Ghnv = gf, hnf
                    else:
                        Gv, Ghnv = G0, Ghn0
                    # per H-half chains: r -> tn -> tn2 -> tanh -> w1 -> h.
                    # z / u=z*h / vm=z-1 run in the shadow; ACT only does
                    # sigmoids+tanh (vm on DVE):  h = u - vm*n.
                    nh = HALVES
                    hw_ = H // nh
                    r_s = work.tile([BL, H], f32, tag="r")
                    z_s = work.tile([BL, H], f32, tag="z")
                    tn = work.tile([BL, H], f32, tag="tn")
                    tn2 = work.tile([BL, H], f32, tag="tn2")
                    n_t = work.tile([BL, H], f32, tag="n")
                    u = work.tile([BL, H], f32, tag="u")
                    vm = work.tile([BL, H], f32, tag="vm")
                    w1 = work.tile([BL, H], f32, tag="w1")
                    sl = [slice(i * hw_, (i + 1) * hw_) for i in range(nh)]
                    for s in sl:
                        nc.scalar.activation(r_s[:, s], Gv[:, s.start : s.stop],
                                             AF.Sigmoid)
                    for s in sl:
                        nc.vector.tensor_mul(tn[:, s], r_s[:, s], Ghnv[:, s])
                        nc.vector.tensor_add(tn2[:, s], tn[:, s], Gin[:, s])
                    for s in sl:
                        nc.scalar.activation(
                            z_s[:, s], Gv[:, H + s.start : H + s.stop],
                            AF.Sigmoid)
                    for s in sl:
                        nc.scalar.activation(n_t[:, s], tn2[:, s], AF.Tanh)
                    for s in sl:
                        if t > 0:
                            nc.vector.tensor_mul(u[:, s], z_s[:, s],
                                                 h_prev[:, s])  # z*h
                        nc.vector.tensor_scalar_sub(vm[:, s], z_s[:, s],
                                                    1.0)  # z-1
                    for s in sl:
                        nc.vector.tensor_mul(w1[:, s], vm[:, s],
                                             n_t[:, s])  # (z-1)*n
                        if t > 0:
                            nc.vector.tensor_sub(h_new[:, s], u[:, s],
                                                 w1[:, s])  # z*h+(1-z)*n
                        else:
                            nc.vector.tensor_scalar_mul(h_new[:, s],
                                                        w1[:, s], -1.0)
                else:
                    rz = work.tile([BL, H2], f32, tag="rz")
                    nc.scalar.activation(rz[:], G0[:], AF.Sigmoid)
                    tn = work.tile([BL, H], f32, tag="tn")
                    nc.vector.tensor_mul(tn[:], rz[:, 0:H], Ghn0[:])
                    tn2 = work.tile([BL, H], f32, tag="tn2")
                    nc.vector.tensor_add(tn2[:], tn[:], Gin[:])
                    n_t = work.tile([BL, H], f32, tag="n")
                    nc.scalar.activation(n_t[:], tn2[:], AF.Tanh)
                    if t == 0:
                        v = work.tile([BL, H], f32, tag="d")
                        nc.scalar.activation(v[:], rz[:, H:H2], AF.Copy,
                                             bias=1.0, scale=-1.0)  # 1 - z
                        nc.vector.tensor_mul(h_new[:], v[:], n_t[:])
                    else:
                        d = work.tile([BL, H], f32, tag="d")
                        nc.vector.tensor_sub(d[:], h_prev[:], n_t[:])
                        m = work.tile([BL, H], f32, tag="m")
                        nc.vector.tensor_mul(m[:], rz[:, H:H2], d[:])
                        nc.vector.tensor_add(h_new[:], n_t[:], m[:])
                h_prev = h_new

            def emit_fc_head():
                nonlocal h_prev, hT
                hT = transpose_h(h_prev, hT_dt=f32r)
                fc1 = gpsum.tile([BL, F1], f32, tag="G")
                mm(fc1[:], ones[:], bfc1[:], start=True, stop=False)
                for k in range(KC):
                    mm(fc1[:], hT[:, k * BL : (k + 1) * BL],
                       wfc1T[:, k * F1 : (k + 1) * F1],
                       start=False, stop=(k == KC - 1))
                o1 = work.tile([BL, F1], f32, tag="o1")
                nc.scalar.activation(o1[:], fc1[:], AF.Relu)

                T2 = tpsum.tile([128, 2 * BL], f32, tag="T")
                nc.tensor.transpose(T2[:, 0:BL], o1[:, 0:128], ident[:])
                nc.tensor.transpose(T2[:, BL : 2 * BL], o1[:, 128:256], ident[:])
                o1T = work.tile([128, 2 * BL], f32r, tag="o1T")
                nc.vector.tensor_copy(o1T[:], T2[:])

                fc2 = gpsum.tile([BL, C], f32, tag="Gin")
                mm(fc2[:], ones[:], bfc2[:], start=True, stop=False)
                mm(fc2[:], o1T[:, 0:BL], wfc2T[:, 0:C], start=False, stop=False)
                mm(fc2[:], o1T[:, BL : 2 * BL], wfc2T[:, C : 2 * C],
                   start=False, stop=True)
                lo = work.tile([BL, C], f32, tag="lo")
                nc.vector.tensor_copy(lo[:], fc2[:])
                nc.sync.dma_start(out=out_d.ap(), in_=lo[:])

            # bench mode: repeat the whole computation in a HW loop so the
            # per-iteration time can be extracted from noisy wall-clock.
            if reps > 1:
                with tc.For_i(0, reps, 1):
                    emit_body()
            else:
                emit_body()

    nc.compile()
    return nc


def _prep_shared(w_ih, w_hh, b_ih, b_hh, w_fc1, b_fc1, w_fc2, b_fc2):
    f = np.float32

    def kmajor(wT, kc, n):  # [kc*128, n] -> [128, kc*n]
        return np.ascontiguousarray(
            wT.reshape(kc, 128, n).transpose(1, 0, 2).reshape(128, kc * n)
        ).astype(f, copy=False)

    whhT = kmajor(np.ascontiguousarray(w_hh.T), KC, G3)
    wihT = np.ascontiguousarray(w_ih.T).astype(f, copy=False)
    b_sum = (b_ih + b_hh).astype(f)
    if BF16H:
        import ml_dtypes
        whhT = whhT.astype(ml_dtypes.bfloat16)
    shared = {
        "whhT": whhT,
        "wihT": wihT,
        "brz": np.ascontiguousarray(b_sum[None, :H2]),
        "bin": np.ascontiguousarray(b_ih.astype(f)[None, H2:G3]),
        "bhn": np.ascontiguousarray(b_hh.astype(f)[None, H2:G3]),
        "ones": np.ones((1, BL), f),
        "wfc1T": kmajor(np.ascontiguousarray(w_fc1.T), KC, F1),
        "bfc1": np.ascontiguousarray(b_fc1.astype(f)[None, :]),
        "wfc2T": kmajor(np.ascontiguousarray(w_fc2.T), 2, C),
        "bfc2": np.ascontiguousarray(b_fc2.astype(f)[None, :]),
    }
    return shared


def _prep_in_maps(inputs):
    x = np.asarray(inputs["x"], dtype=np.float32)
    shared = _prep_shared(
        *(np.asarray(inputs[k], dtype=np.float32)
          for k in ("w_ih", "w_hh", "b_ih", "b_hh", "w_fc1", "b_fc1",
                    "w_fc2", "b_fc2"))
    )
    in_maps = []
    for c in range(NCORES):
        xs = x[c * BL : (c + 1) * BL]  # [64, T, I]
        xT = np.ascontiguousarray(xs.transpose(2, 1, 0).reshape(128, T * BL))
        in_maps.append({**shared, "xT": xT})
    return in_maps


def _execute(in_maps, reps=1):
    from concourse.bass_utils import run_bass_kernel_spmd

    key = ("nc", reps)
    if key not in _CACHE:
        _CACHE[key] = _build_program(reps=reps)
    nc = _CACHE[key]
    res = run_bass_kernel_spmd(nc, in_maps, core_ids=list(range(NCORES)))
    out = np.concatenate([res.results[c]["logits"] for c in range(NCORES)], axis=0)
    return out.astype(np.float32), res


def _run(inputs, trace=False, trace_kwargs=None):
    return _execute(_prep_in_maps(inputs))


def kernel(**inputs):
    out, _ = _execute(_prep_in_maps(inputs))
    return out

